# revision 1
# baseline (speedup 1.0000x reference)
"""GCN message-passing kernel for Trainium2 (8 NeuronCores).

Problem: x [4,4096,64] f32, graph [4,4096,4096] f32, W [64,256], b [64].
  g = graph + I;  d = 1/(sqrt(g.sum(-1)) + 1e-7);  A = D g D
  h_{k+1} = A h_k (3 layers);  out = concat([x,h1,h2,h3], -1) @ W.T + b

Strategy (all sizes hardcoded):
  - 4 groups of 2 cores; group g handles batch element g; each core owns
    2048 graph rows.  Host pre-adds self loops, casts the shard to fp16,
    lays it out transposed + p-major, and PERMUTES the contraction axis
    per core to [own nodes | peer nodes] so own-half matmuls never wait
    on a collective.
  - Normalization d host-precomputed; device works in u-space
    (u_{k+1} = d^2 * (g @ u_k)).
  - Matmuls: lhsT = g^T tile [128j,128i], rhs = u j-tile [128j,64].
  - Layer 1 accumulates while the shard streams (i-chunked so the first
    u1 chunk AllGathers under the tail of the load).
  - Peer halves of u1/u2 land via AllGather + dma_gather with a per-core
    host index (peer-rank row block) - no control flow, SPMD-uniform.
  - Layers 2/3 split into own-phase (local u, overlaps the collectives)
    and peer-phase (after the gather).
  - Final linear folds 1/d: out = (1/d) * (d (x) b + ucat @ W.T),
    software-pipelined into layer 3's peer phase.
"""

import sys

for _p in ("/opt/trn_rl_repo", "/opt/pypackages"):
    if _p not in sys.path:
        sys.path.insert(0, _p)

import numpy as np

import concourse.bass as bass
import concourse.mybir as mybir
from concourse import tile
from concourse.bass_utils import run_bass_kernel_spmd

F32 = mybir.dt.float32
F16 = mybir.dt.float16

B = 4          # batch
N = 4096       # nodes
D = 64         # feature dim
DEPTH = 3
NCORES = 8
ROWS = N // 2          # rows (output nodes) per core
JT = N // 128          # 32 contraction (j) tiles
IT = ROWS // 128       # 16 own row (i) tiles per core
NA = 9                 # i-tiles in the early exchange chunk
CA = NA * 128

_MAX_DRAIN_WAITS = 1   # this walrus build encodes at most 1 sem-wait per CTRL inst


def _split_drain_waits(nc):
    """This walrus build encodes at most one sem-wait per instruction for
    several instruction structs; hoist excess waits onto injected
    same-engine Drain instructions placed immediately before."""
    for bb in nc.main_func.blocks:
        il = bb.instructions  # live list
        i = 0
        while i < len(il):
            ins = il[i]
            si = getattr(ins, "sync_info", None)
            if (si is not None and getattr(ins, "engine", None) is not None
                    and len(si.on_wait) > _MAX_DRAIN_WAITS):
                waits = list(si.on_wait)
                pre = []
                k = 0
                while len(waits) - k > _MAX_DRAIN_WAITS:
                    chunk = waits[k:k + _MAX_DRAIN_WAITS]
                    k += _MAX_DRAIN_WAITS
                    pre.append(mybir.InstDrain(
                        name=f"{ins.name}-sw{len(pre)}",
                        opcode="Drain",
                        engine=ins.engine,
                        debug=ins.debug,
                        ins=[], outs=[],
                        sync_info=mybir.SyncInfo(on_wait=chunk, on_update=[]),
                    ))
                ins.sync_info = mybir.SyncInfo(
                    on_wait=waits[k:], on_update=list(si.on_update))
                for j, d in enumerate(pre):
                    il.insert(i + j, d)
                i += len(pre)
            i += 1


def _build_program():
    nc = bass.Bass(trn_type="TRN2", num_devices=NCORES)

    # graph^T shard, fp16, +I, p-major, j-axis permuted [own | peer]:
    # tg[p, jt*ROWS + i] = (graph[g]+I)[rows[i], jorder[jt*128+p]]
    tg = nc.dram_tensor("tg", [128, JT * ROWS], F16, kind="ExternalInput")
    # u0 = d*x, j-permuted per core, p-major
    u0d = nc.dram_tensor("u0d", [128, JT * D], F16, kind="ExternalInput")
    u0t = nc.dram_tensor("u0t", [D, ROWS], F16, kind="ExternalInput")
    dcol = nc.dram_tensor("dcol", [1, ROWS], F16, kind="ExternalInput")
    ei = nc.dram_tensor("ei", [128, IT], F32, kind="ExternalInput")
    ri = nc.dram_tensor("ri", [128, IT], F32, kind="ExternalInput")
    bvec = nc.dram_tensor("bvec", [1, D], F16, kind="ExternalInput")
    wt = nc.dram_tensor("wt", [2, 128, D], F16, kind="ExternalInput")
    ident = nc.dram_tensor("ident", [128, 128], F16, kind="ExternalInput")
    out = nc.dram_tensor("out", [128, IT * D], F32, kind="ExternalOutput")

    groups = [[2 * g, 2 * g + 1] for g in range(B)]

    with tile.TileContext(nc) as tc:
        with tc.tile_pool(name="res", bufs=1) as res_pool, \
             tc.tile_pool(name="small", bufs=1) as small_pool, \
             tc.tile_pool(name="ubuf", bufs=1) as u_pool, \
             tc.tile_pool(name="psacc", bufs=1, space="PSUM") as psacc, \
             tc.tile_pool(name="pssm", bufs=2, space="PSUM") as pssm, \
             tc.tile_pool(name="outp", bufs=1) as out_pool, \
             tc.tile_pool(name="dram", bufs=1, space="DRAM") as dram_pool:

            # e first (layer-1 scales gate on it), then u0 (matmuls gate on it)
            e_sb = small_pool.tile([128, IT], F32, tag="esb")
            nc.sync.dma_start(e_sb[:], ei[:])
            u0_sb = u_pool.tile([128, JT * D], F16, tag="u0", name="u0_sb")
            nc.sync.dma_start(u0_sb[:], u0d[:])

            resident = res_pool.tile([128, JT * ROWS], F16, tag="resident")
            psA = psacc.tile([128, 512], F32, tag="psA", name="psA")
            psB = psacc.tile([128, 512], F32, tag="psB", name="psB")

            def ps_slice(it):
                t = psA if it < 8 else psB
                k = it % 8
                return t[:, k * D:(k + 1) * D]

            def load_chunk(h, jt):
                """DMA the (i-chunk h, j-tile jt) block and fold it into L1."""
                lo, hi = (0, CA) if h == 0 else (CA, ROWS)
                c0 = jt * ROWS + lo
                rslice = resident[:, c0:c0 + (hi - lo)]
                nc.sync.dma_start(rslice, tg[:, c0:c0 + (hi - lo)])
                for k in range((hi - lo) // 128):
                    it = lo // 128 + k
                    nc.tensor.matmul(
                        ps_slice(it),
                        rslice[:, k * 128:(k + 1) * 128],
                        u0_sb[:, jt * D:(jt + 1) * D],
                        start=(jt == 0), stop=(jt == JT - 1),
                    )

            def scale_tile(layer, it, u_own):
                """u_{k+1}[it] = e * ps[it] (fp16)."""
                usl = u_own[:, it * D:(it + 1) * D]
                if it % 2 == 0:
                    nc.vector.tensor_scalar_mul(usl, ps_slice(it),
                                                e_sb[:, it:it + 1])
                else:
                    nc.scalar.activation(usl, ps_slice(it),
                                         mybir.ActivationFunctionType.Copy,
                                         scale=e_sb[:, it:it + 1])

            def trans_tile(layer, it, u_own, cat_dst, roff):
                """cat rows <- u_{k+1}[it]^T (PE transpose + copy out)."""
                usl = u_own[:, it * D:(it + 1) * D]
                ps_tr = pssm.tile([D, 128], F16, tag="tr", name=f"tr{layer}_{it}")[:]
                nc.tensor.transpose(ps_tr, usl, id_f16[:])
                dst = cat_dst[roff:roff + D, it * 128:(it + 1) * 128]
                if it % 2 == 0:
                    nc.scalar.copy(dst, ps_tr)
                else:
                    nc.vector.tensor_copy(dst, ps_tr)

            # i-chunk A streams first
            load_chunk(0, 0)
            load_chunk(0, 1)

            id_f16 = small_pool.tile([128, 128], F16, tag="idf16")
            nc.sync.dma_start(id_f16[:], ident[:])

            u1_own = u_pool.tile([128, IT * D], F16, tag="uown", name="u1_own")
            u2_own = u_pool.tile([128, IT * D], F16, tag="uown2", name="u2_own")
            u3_own = u_pool.tile([128, IT * D], F16, tag="uown3", name="u3_own")
            ugp1 = u_pool.tile([128, IT * D], F16, tag="ugp1", name="ugp1")
            ugp2 = u_pool.tile([128, IT * D], F16, tag="ugp2", name="ugp2")

            catA = small_pool.tile([128, ROWS], F16, tag="catA")
            catB = small_pool.tile([128, ROWS], F16, tag="catB")

            for jt in range(2, JT):
                load_chunk(0, jt)
            for it in range(NA):
                scale_tile(1, it, u1_own)

            def send_chunk(u_own, lo, hi, xtag):
                """ReduceScatter of [u_own | u_own]: every rank receives
                u(0)+u(1) (rank-symmetric); peer half = sum - u_own."""
                w = (hi - lo) * D
                snd = dram_pool.tile([256, w], F16, name=f"snd{xtag}",
                                     tag=f"snd{xtag}")
                rcv = dram_pool.tile([128, w], F16, name=f"rcv{xtag}",
                                     tag=f"rcv{xtag}")
                nc.sync.dma_start(snd[0:128, :], u_own[:, lo * D:hi * D])
                nc.sync.dma_start(snd[128:256, :], u_own[:, lo * D:hi * D])
                nc.gpsimd.collective_compute(
                    "ReduceScatter", mybir.AluOpType.add,
                    replica_groups=groups,
                    ins=[snd[:].opt()], outs=[rcv[:].opt()])
                return rcv

            rcvA = send_chunk(u1_own, 0, NA, "1a")

            def mm(it, jt, rhs, start, stop):
                nc.tensor.matmul(
                    ps_slice(it),
                    resident[:, jt * ROWS + it * 128: jt * ROWS + (it + 1) * 128],
                    rhs, start=start, stop=stop)

            # L2 own-phase head: chunk-A u1 tiles x chunk-A i-tiles are ready
            # (and their PSUM slices free) while i-chunk B is still loading
            for jj, jt in enumerate(range(NA)):
                for it in range(NA):
                    mm(it, jt, u1_own[:, jt * D:(jt + 1) * D], jj == 0, False)

            # ---- i-chunk B load + remaining layer-1 ----
            for jt in range(JT):
                load_chunk(1, jt)
            for it in range(NA, IT):
                scale_tile(1, it, u1_own)
            rcvB = send_chunk(u1_own, NA, IT, "1b")
            for it in range(NA):
                trans_tile(1, it, u1_own, catA, D)
            # late constants (final linear only)
            wt_sb = small_pool.tile([128, 2 * D], F16, tag="wt")
            nc.sync.dma_start(wt_sb[:, 0:D], wt[0])
            nc.sync.dma_start(wt_sb[:, D:2 * D], wt[1])
            b_sb = small_pool.tile([1, D], F16, tag="bsb")
            nc.sync.dma_start(b_sb[:], bvec[:])
            d_sb = small_pool.tile([1, ROWS], F16, tag="dsb")
            nc.sync.dma_start(d_sb[:], dcol[:])
            r_sb = small_pool.tile([128, IT], F32, tag="rsb")
            nc.sync.dma_start(r_sb[:], ri[:])
            nc.sync.dma_start(catA[0:D, :], u0t[:])
            for it in range(NA, IT):
                trans_tile(1, it, u1_own, catA, D)

            uscr = u_pool.tile([128, IT * D], F16, tag="uscr", name="uscr")

            def recv_peer(dst, rcv, u_own, lo, hi, scol):
                """dst = (u(0)+u(1)) - u_own over it-tiles [lo, hi)."""
                w = (hi - lo) * D
                ssl = uscr[:, scol:scol + w]
                nc.sync.dma_start(ssl, rcv[:])
                nc.vector.tensor_tensor(dst, ssl, u_own[:, lo * D:hi * D],
                                        mybir.AluOpType.subtract)

            recv_peer(ugp1[:, 0:NA * D], rcvA, u1_own, 0, NA, 0)

            # ---- layer 2 own phase remainder ----
            # chunk-A jts for the late i-tiles (their groups start here)
            for jj, jt in enumerate(range(NA)):
                for it in range(NA, IT):
                    mm(it, jt, u1_own[:, jt * D:(jt + 1) * D], jj == 0, False)
            # chunk-B jts for every i-tile
            for jt in range(NA, IT):
                for it in range(IT):
                    mm(it, jt, u1_own[:, jt * D:(jt + 1) * D], False, False)
            # peer chunk A (slots 16..16+NA-1)
            for jt in range(NA):
                for it in range(IT):
                    mm(it, 16 + jt, ugp1[:, jt * D:(jt + 1) * D], False, False)
            recv_peer(ugp1[:, NA * D:IT * D], rcvB, u1_own, NA, IT, NA * D)
            # peer chunk B (slots 16+NA..31), it-outer, close + scale
            for it in range(IT):
                for jj, jt in enumerate(range(NA, IT)):
                    mm(it, 16 + jt, ugp1[:, jt * D:(jt + 1) * D], False,
                       jj == (IT - NA) - 1)
                scale_tile(2, it, u2_own)

            # ---- u2 exchange (one shot) ----
            rcv2 = send_chunk(u2_own, 0, IT, "2")

            # ---- layer 3 own phase (overlaps RS2) ----
            for jj, jt in enumerate(range(IT)):
                for it in range(IT):
                    mm(it, jt, u2_own[:, jt * D:(jt + 1) * D], jj == 0, False)
            for it in range(IT):
                trans_tile(2, it, u2_own, catB, 0)

            # final-linear pass 1 (d (x) b + catA part) also fills the RS2 window
            o1_sb = out_pool.tile([128, IT * D], F32, tag="o1sb")
            for it in range(IT):
                ps_p = pssm.tile([128, D], F32, tag="fin", bufs=4,
                                 name=f"psp{it}")[:]
                isl = slice(it * 128, (it + 1) * 128)
                nc.tensor.matmul(ps_p, d_sb[0:1, isl], b_sb[:],
                                 start=True, stop=False)
                nc.tensor.matmul(ps_p, catA[:, isl], wt_sb[:, 0:D],
                                 start=False, stop=True)
                o1l = o1_sb[:, it * D:(it + 1) * D]
                if it % 2 == 0:
                    nc.vector.tensor_scalar_mul(o1l, ps_p, r_sb[:, it:it + 1])
                else:
                    nc.scalar.activation(o1l, ps_p,
                                         mybir.ActivationFunctionType.Copy,
                                         scale=r_sb[:, it:it + 1])

            recv_peer(ugp2[:], rcv2, u2_own, 0, IT, 0)

            # ---- layer 3 peer phase + final, software-pipelined ----
            o_sb = out_pool.tile([128, IT * D], F32, tag="osb")

            def final_group(it):
                ps_o = pssm.tile([128, D], F32, tag="fin", bufs=4,
                                 name=f"pso{it}")[:]
                isl = slice(it * 128, (it + 1) * 128)
                nc.tensor.matmul(ps_o, catB[:, isl], wt_sb[:, D:2 * D],
                                 start=True, stop=True)
                osl = o_sb[:, it * D:(it + 1) * D]
                o1l = o1_sb[:, it * D:(it + 1) * D]
                if it % 2 == 0:
                    nc.vector.tensor_scalar_mul(osl, ps_o, r_sb[:, it:it + 1])
                else:
                    nc.scalar.activation(osl, ps_o,
                                         mybir.ActivationFunctionType.Copy,
                                         scale=r_sb[:, it:it + 1])
                nc.vector.tensor_tensor(osl, osl, o1l, mybir.AluOpType.add)

            for it in range(IT):
                for jj, jt in enumerate(range(IT)):
                    mm(it, 16 + jt, ugp2[:, jt * D:(jt + 1) * D], False,
                       jj == IT - 1)
                scale_tile(3, it, u3_own)
                if it >= 2:
                    trans_tile(3, it - 2, u3_own, catB, D)
                if it >= 4:
                    final_group(it - 4)
            trans_tile(3, IT - 2, u3_own, catB, D)
            trans_tile(3, IT - 1, u3_own, catB, D)
            nc.sync.dma_start(out[:, 0:(IT - 4) * D], o_sb[:, 0:(IT - 4) * D])
            for k in (4, 3, 2, 1):
                final_group(IT - k)
            nc.sync.dma_start(out[:, (IT - 4) * D:], o_sb[:, (IT - 4) * D:])

    _split_drain_waits(nc)
    return nc


_NC_CACHE = None


def _get_program():
    global _NC_CACHE
    if _NC_CACHE is None:
        _NC_CACHE = _build_program()
    return _NC_CACHE


def _prep_inputs(x, graph, W, b):
    wt_h = np.ascontiguousarray(W.T.reshape(2, 128, D)).astype(np.float16)
    b_h = np.ascontiguousarray(b.reshape(1, D)).astype(np.float16)
    ident = np.eye(128, dtype=np.float16)

    in_maps = []
    for g in range(B):
        gg = graph[g] + np.eye(N, dtype=np.float32)
        dg = 1.0 / (np.sqrt(gg.sum(axis=1)) + 1e-7)
        u0g = (dg[:, None] * x[g]).astype(np.float16)
        g16 = gg.astype(np.float16)
        for r in range(2):
            rows = slice(r * ROWS, (r + 1) * ROWS)
            own = np.arange(r * ROWS, (r + 1) * ROWS)
            peer = np.arange((1 - r) * ROWS, (2 - r) * ROWS)
            jorder = np.concatenate([own, peer])
            # tg[p, jt*ROWS+i] = gg[rows[i], jorder[jt*128+p]]
            tgc = g16[rows, :][:, jorder].T               # [N(perm), ROWS]
            tg_h = np.ascontiguousarray(
                tgc.reshape(JT, 128, ROWS).transpose(1, 0, 2).reshape(128, JT * ROWS))
            u0p = u0g[jorder]                             # [N, D] permuted
            u0d_h = np.ascontiguousarray(
                u0p.reshape(JT, 128, D).transpose(1, 0, 2).reshape(128, JT * D))
            u0t_h = np.ascontiguousarray(u0g[rows, :].T)  # [D, ROWS]
            d_own = dg[rows]
            dcol_h = np.ascontiguousarray(d_own.reshape(1, ROWS)).astype(np.float16)
            ei_h = np.ascontiguousarray(
                (d_own * d_own).reshape(IT, 128).T).astype(np.float32)
            ri_h = np.ascontiguousarray(
                (1.0 / d_own).reshape(IT, 128).T).astype(np.float32)
            in_maps.append({"tg": tg_h, "u0d": u0d_h, "u0t": u0t_h,
                            "dcol": dcol_h, "ei": ei_h, "ri": ri_h,
                            "bvec": b_h, "wt": wt_h, "ident": ident})
    return in_maps


def kernel(x, graph, W, b, trace=False, **kw):
    nc = _get_program()
    in_maps = _prep_inputs(np.asarray(x, np.float32), np.asarray(graph, np.float32),
                           np.asarray(W, np.float32), np.asarray(b, np.float32))
    res = run_bass_kernel_spmd(nc, in_maps, core_ids=list(range(NCORES)),
                               trace=trace, **kw)
    out = np.empty((B, N, D), np.float32)
    for c in range(NCORES):
        g, r = divmod(c, 2)
        o = res.results[c]["out"]
        out[g, r * ROWS:(r + 1) * ROWS, :] = (
            o.reshape(128, IT, D).transpose(1, 0, 2).reshape(ROWS, D))
    if trace:
        kernel.last_exec_time_ns = res.exec_time_ns
        kernel.last_results = res
    return out

